# revision 18
# baseline (speedup 1.0000x reference)
"""Trainium2 Bass kernel for GQA attention (nn_Attention_43181601194655).

Full module: hidden [B,S,HID] -> Wq/Wk/Wv proj -> RoPE -> causal GQA attention
-> Wo proj. Sharded tensor-parallel over heads across 8 NeuronCores:
core c owns q-heads [4c..4c+4) and kv-head c (Wq/Wk/Wv column slices, Wo row
slice). Each core computes a full-shape partial output; the host sums the 8
partials (the row-parallel Wo reduction).

v2 pipeline (single pass over q supertiles, per batch):
  for each 512-wide s-supertile ss:
    project Q^T/K^T/V^T for ss (hidden^T streamed in bf16), RoPE on-chip
    PE-transpose V^T -> V tiles (with ones column for softmax denominators)
    attention for q-super ss (all 4 heads, keys 0..ss complete by now):
      S^T[k,q] matmuls head-paired on PE row groups (bases 0/64), exp on
      ScalarE straight from PSUM with fused 1/sqrt(D) scale -> P^T bf16
      PV as lhsT=V_aug (stationary), rhs=P^T wide-N, causal-trimmed column
      ranges accumulating in PSUM; row 64 = softmax denominator
      normalize along q: DVE reciprocal -> GpSimd partition_broadcast ->
      one DVE multiply writing attn^T[hd, q] bf16
    Wo matmuls for ss's four 128-row output tiles + DMA out fp32 partial
"""

import sys

if "/opt/trn_rl_repo" not in sys.path:
    sys.path.insert(0, "/opt/trn_rl_repo")

import numpy as np
import ml_dtypes

import concourse.bass as bass
from concourse import bacc
import concourse.mybir as mybir
from concourse.tile import TileContext
from concourse.masks import make_identity

BF16 = mybir.dt.bfloat16
F32 = mybir.dt.float32

B, S, HID = 2, 2048, 2048
H, HKV, D = 32, 8, 64
NCORES = 8
HQ = H // NCORES          # q heads per core (4)
HD = HQ * D               # 256: per-core attn feature dim
SCALE = D ** -0.5
SSUP = 512                # q supertile width
NEG = -1e9


def build_nc(b_sz=B, s_sz=S, hid=HID):
    """Build the per-core Bass program. Parameterized for small-sim testing."""
    C = hid // 128            # contraction chunks
    n_st = s_sz // 128        # 128-tiles along s
    sup = min(SSUP, s_sz)
    n_sup = s_sz // sup
    n_qt = sup // 128         # q-tiles per supertile
    n_cs = hid // 512         # 512-wide output column chunks

    nc = bacc.Bacc()
    hsT = nc.dram_tensor("hsT", [hid, b_sz * s_sz], BF16, kind="ExternalInput")
    wq = nc.dram_tensor("wq", [hid, HQ * D], BF16, kind="ExternalInput")
    wkv = nc.dram_tensor("wkv", [hid, 128], BF16, kind="ExternalInput")
    wo = nc.dram_tensor("wo", [HD, hid], BF16, kind="ExternalInput")
    cos2 = nc.dram_tensor("cos2", [128, s_sz], F32, kind="ExternalInput")
    sinx = nc.dram_tensor("sinx", [128, s_sz], F32, kind="ExternalInput")
    maskd = nc.dram_tensor("maskd", [128, 128], F32, kind="ExternalInput")
    out = nc.dram_tensor("out", [b_sz * s_sz, hid], F32, kind="ExternalOutput")

    hsT_v = hsT.rearrange("(co p) n -> p co n", p=128)
    wq_v = wq.rearrange("(co p) m -> p co m", p=128)
    wkv_v = wkv.rearrange("(co p) m -> p co m", p=128)
    wo_v = wo.rearrange("(j p) n -> p j n", p=128)

    with TileContext(nc) as tc:
        with (
            tc.tile_pool(name="const", bufs=1) as cpool,
            tc.tile_pool(name="hst", bufs=2) as hpool,
            tc.tile_pool(name="perb", bufs=2) as bpool,
            tc.tile_pool(name="pt", bufs=6) as ptpool,
            tc.tile_pool(name="work", bufs=2) as wpool,
            tc.tile_pool(name="outsb", bufs=2) as opool,
            tc.tile_pool(name="psum_qk", bufs=5, space="PSUM") as qkpool,
            tc.tile_pool(name="psum_pv", bufs=2, space="PSUM") as pvpool,
            tc.tile_pool(name="psum_tp", bufs=1, space="PSUM") as tppool,
        ):
            # ---- constants ----
            wq_t = cpool.tile([128, C, HQ * D], BF16, tag="wq")
            nc.sync.dma_start(wq_t[:], wq_v[:])
            wkv_t = cpool.tile([128, C, 128], BF16, tag="wkv")
            nc.sync.dma_start(wkv_t[:], wkv_v[:])
            wo_t = cpool.tile([128, HD // 128, hid], BF16, tag="wo")
            nc.sync.dma_start(wo_t[:], wo_v[:])
            cos_t = cpool.tile([128, s_sz], F32, tag="cos")
            nc.sync.dma_start(cos_t[:], cos2[:])
            sin_t = cpool.tile([128, s_sz], F32, tag="sin")
            nc.sync.dma_start(sin_t[:], sinx[:])
            mask_t = cpool.tile([128, 128], F32, tag="mask")
            nc.sync.dma_start(mask_t[:], maskd[:])
            ident = cpool.tile([128, 128], BF16, tag="ident")
            make_identity(nc, ident[:])

            def rope(dst, psum, s0, w, rows):
                """dst[bf16 SBUF [rows,w]] = RoPE(psum[:rows,:w]), tables at
                cols s0:s0+w. rows=64 (K) or 128 (2 stacked q-heads);
                out = psum*cos + shift32(psum)*sinx (sinx sign-folded)."""
                u = wpool.tile([128, sup], F32, tag="rope_u")
                t = wpool.tile([128, sup], F32, tag="rope_t")
                nc.vector.tensor_tensor(
                    u[:rows, :w], psum[:rows, :w], cos_t[:rows, s0:s0 + w],
                    mybir.AluOpType.mult)
                for o in range(0, rows, 64):
                    nc.vector.tensor_tensor(
                        t[o:o + 32, :w], psum[o + 32:o + 64, :w],
                        sin_t[o:o + 32, s0:s0 + w], mybir.AluOpType.mult)
                    nc.vector.tensor_tensor(
                        t[o + 32:o + 64, :w], psum[o:o + 32, :w],
                        sin_t[o + 32:o + 64, s0:s0 + w], mybir.AluOpType.mult)
                nc.vector.tensor_tensor(
                    dst, u[:rows, :w], t[:rows, :w], mybir.AluOpType.add)

            for b in range(b_sz):
                qt_b = bpool.tile([128, HQ // 2, s_sz], BF16, tag="qt")
                # K^T duplicated in both partition halves so paired QK
                # matmuls run in distinct PE row groups (bases 0 and 64).
                kt_b = bpool.tile([128, s_sz], BF16, tag="kt")
                vt_b = bpool.tile([64, s_sz], BF16, tag="vt")
                v_b = bpool.tile([128, n_st, 72], BF16, tag="v")
                attnT_b = bpool.tile([128, HD // 128, s_sz], BF16, tag="attnT")
                nc.vector.memset(v_b[:, :, 64:65], 1.0)

                # ---- projections (all supertiles first: RoPE latency is off
                # the attention critical path, attention(qs) can start as soon
                # as super qs is projected) ----
                for ss in range(n_sup):
                    s0 = ss * sup
                    nbase = b * s_sz + s0
                    hst = hpool.tile([128, C, sup], BF16, tag="hst")
                    nc.sync.dma_start(hst[:], hsT_v[:, :, nbase:nbase + sup])
                    psqs = [qkpool.tile([128, sup], F32, tag="qk",
                                        name=f"psq{i}") for i in range(2)]
                    for cc in range(C):
                        for hp in range(HQ // 2):
                            nc.tensor.matmul(
                                psqs[hp][:],
                                wq_t[:, cc, hp * 128:(hp + 1) * 128],
                                hst[:, cc, :],
                                start=(cc == 0), stop=(cc == C - 1))
                    pskv = qkpool.tile([128, sup], F32, tag="qk")
                    for cc in range(C):
                        nc.tensor.matmul(
                            pskv[:], wkv_t[:, cc, :], hst[:, cc, :],
                            start=(cc == 0), stop=(cc == C - 1))
                    for hp in range(HQ // 2):
                        rope(qt_b[:, hp, s0:s0 + sup], psqs[hp], s0, sup, 128)
                    rope(kt_b[:64, s0:s0 + sup], pskv[:], s0, sup, 64)
                    nc.vector.tensor_copy(
                        kt_b[64:128, s0:s0 + sup], kt_b[:64, s0:s0 + sup])
                    nc.vector.tensor_copy(
                        vt_b[:, s0:s0 + sup], pskv[64:128, :])
                    for st4 in range(n_qt):
                        st = ss * n_qt + st4
                        pst = tppool.tile([128, 128], BF16, tag="tp")
                        nc.tensor.transpose(
                            pst[:, :64], vt_b[:, st * 128:(st + 1) * 128],
                            ident[:64, :64])
                        nc.vector.tensor_copy(v_b[:, st, :64], pst[:, :64])

                # ---- attention ----
                for ss in range(n_sup):
                    s0 = ss * sup
                    n_kt = (ss + 1) * n_qt
                    for hp in range(HQ // 2):
                        heads = (2 * hp, 2 * hp + 1)
                        psvs = [pvpool.tile([128, sup], F32, tag="pv",
                                            name=f"psv{i}")
                                for i in range(2)]
                        for kt in range(n_kt):
                            k0 = kt * 128
                            dq = max(0, k0 - s0)
                            w = sup - dq
                            pt = ptpool.tile([128, 2, sup], BF16, tag="pt")
                            for sub, h in enumerate(heads):
                                o = (h % 2) * 64
                                qh = qt_b[o:o + 64, h // 2, :]
                                kth = kt_b[o:o + 64, :]
                                ps = qkpool.tile([128, sup], F32, tag="qk")
                                if k0 < s0:
                                    nc.tensor.matmul(
                                        ps[:], kth[:, k0:k0 + 128],
                                        qh[:, s0:s0 + sup],
                                        start=True, stop=True)
                                else:
                                    nc.tensor.matmul(
                                        ps[:, 0:128],
                                        kth[:, k0:k0 + 128],
                                        qh[:, k0:k0 + 128],
                                        start=True, stop=True)
                                    if w > 128:
                                        nc.tensor.matmul(
                                            ps[:, 128:w],
                                            kth[:, k0:k0 + 128],
                                            qh[:, k0 + 128:s0 + sup],
                                            start=True, stop=True)
                                if k0 >= s0:
                                    nc.vector.tensor_tensor(
                                        ps[:, 0:128], ps[:, 0:128], mask_t[:],
                                        mybir.AluOpType.add)
                                nc.scalar.activation(
                                    pt[:, sub, dq:dq + w], ps[:, 0:w],
                                    mybir.ActivationFunctionType.Exp,
                                    scale=SCALE)
                            for sub in range(2):
                                nc.tensor.matmul(
                                    psvs[sub][0:65, dq:sup],
                                    v_b[:, kt, :65],
                                    pt[:, sub, dq:sup],
                                    start=(kt == 0), stop=(kt == n_kt - 1),
                                    skip_group_check=True)
                        for sub, h in enumerate(heads):
                            psv = psvs[sub]
                            # 1/Z along the free dim: DVE reciprocal on a
                            # [1,512] row is single-lane serial (~3.3us), so
                            # compute exp(-ln Z) on ScalarE instead.
                            lnz = wpool.tile([1, sup], F32, tag="lnz")
                            nc.scalar.activation(
                                lnz[:], psv[64:65, :],
                                mybir.ActivationFunctionType.Ln)
                            recip = wpool.tile([1, sup], F32, tag="recip")
                            nc.scalar.activation(
                                recip[:], lnz[:],
                                mybir.ActivationFunctionType.Exp, scale=-1.0)
                            bcast = wpool.tile([64, sup], F32, tag="bcast")
                            nc.gpsimd.partition_broadcast(bcast[:], recip[:])
                            o = (h % 2) * 64
                            nc.vector.tensor_tensor(
                                attnT_b[o:o + 64, h // 2, s0:s0 + sup],
                                psv[0:64, :], bcast[:], mybir.AluOpType.mult)

                # ---- output projection ----
                for st in range(n_st):
                    osb = opool.tile([128, hid], F32, tag="osb")
                    for cs in range(n_cs):
                        pso = qkpool.tile([128, sup], F32, tag="qk")
                        for j in range(HD // 128):
                            nc.tensor.matmul(
                                pso[:, :512],
                                attnT_b[:, j, st * 128:(st + 1) * 128],
                                wo_t[:, j, cs * 512:(cs + 1) * 512],
                                start=(j == 0),
                                stop=(j == HD // 128 - 1))
                        nc.vector.tensor_copy(
                            osb[:, cs * 512:(cs + 1) * 512], pso[:, :512])
                    row = b * s_sz + st * 128
                    nc.sync.dma_start(out[row:row + 128, :], osb[:])
    nc.compile()
    return nc


def _rope_tables_np(seq_len, dim, base=10000.0):
    inv_freq = 1.0 / (base ** (np.arange(0, dim, 2, dtype=np.float32) / dim))
    t = np.arange(seq_len, dtype=np.float32)
    freqs = np.outer(t, inv_freq)
    emb = np.concatenate([freqs, freqs], axis=-1)
    return np.cos(emb), np.sin(emb)


def host_prep(hidden_states, cos, sin, Wq, Wk, Wv, Wo, s_sz=None, hid=None):
    """Slice/transposes/casts -> per-core input maps."""
    b_sz = hidden_states.shape[0]
    s_sz = s_sz or hidden_states.shape[1]
    hid = hid or hidden_states.shape[2]
    bf = ml_dtypes.bfloat16

    hsT = np.ascontiguousarray(
        hidden_states.reshape(b_sz * s_sz, hid).T).astype(bf)

    cosT = np.asarray(cos, np.float32).T          # [64, S]
    sinT = np.asarray(sin, np.float32).T
    cos2 = np.concatenate([cosT, cosT], axis=0)   # [128, S]
    sinx = np.concatenate(
        [-sinT[:32], sinT[32:64], -sinT[:32], sinT[32:64]], axis=0)
    cos2 = np.ascontiguousarray(cos2, dtype=np.float32)
    sinx = np.ascontiguousarray(sinx, dtype=np.float32)

    kk, qq = np.meshgrid(np.arange(128), np.arange(128), indexing="ij")
    maskd = np.where(kk <= qq, 0.0, NEG).astype(np.float32)

    in_maps = []
    for c in range(NCORES):
        wq_c = np.ascontiguousarray(Wq[:, c * HD:(c + 1) * HD]).astype(bf)
        wkv_c = np.concatenate(
            [Wk[:, c * D:(c + 1) * D], Wv[:, c * D:(c + 1) * D]], axis=1
        ).astype(bf)
        wo_c = np.ascontiguousarray(Wo[c * HD:(c + 1) * HD, :]).astype(bf)
        in_maps.append({
            "hsT": hsT, "wq": wq_c, "wkv": np.ascontiguousarray(wkv_c),
            "wo": wo_c, "cos2": cos2, "sinx": sinx, "maskd": maskd,
        })
    return in_maps


def kernel_run(hidden_states, cos, sin, attention_mask, Wq, Wk, Wv, Wo,
               **spmd_kwargs):
    from concourse.bass_utils import run_bass_kernel_spmd

    hidden_states = np.asarray(hidden_states, np.float32)
    in_maps = host_prep(hidden_states, cos, sin,
                        np.asarray(Wq, np.float32), np.asarray(Wk, np.float32),
                        np.asarray(Wv, np.float32), np.asarray(Wo, np.float32))
    nc = build_nc()
    res = run_bass_kernel_spmd(nc, in_maps, core_ids=list(range(NCORES)),
                               **spmd_kwargs)
    acc = np.zeros((B * S, HID), np.float64)
    for r in res.results:
        acc += r["out"].astype(np.float64)
    return acc.reshape(B, S, HID).astype(np.float32), res


def kernel(hidden_states, cos, sin, attention_mask, Wq, Wk, Wv, Wo):
    out, _ = kernel_run(hidden_states, cos, sin, attention_mask,
                        Wq, Wk, Wv, Wo)
    return out


if __name__ == "__main__":
    pass


# revision 19
# speedup vs baseline: 1.2183x; 1.2183x over previous
"""Trainium2 Bass kernel for GQA attention (nn_Attention_43181601194655).

Full module: hidden [B,S,HID] -> Wq/Wk/Wv proj -> RoPE -> causal GQA attention
-> Wo proj. Sharded tensor-parallel over heads across 8 NeuronCores:
core c owns q-heads [4c..4c+4) and kv-head c (Wq/Wk/Wv column slices, Wo row
slice). Each core computes a full-shape partial output; the host sums the 8
partials (the row-parallel Wo reduction).

v2 pipeline (single pass over q supertiles, per batch):
  for each 512-wide s-supertile ss:
    project Q^T/K^T/V^T for ss (hidden^T streamed in bf16), RoPE on-chip
    PE-transpose V^T -> V tiles (with ones column for softmax denominators)
    attention for q-super ss (all 4 heads, keys 0..ss complete by now):
      S^T[k,q] matmuls head-paired on PE row groups (bases 0/64), exp on
      ScalarE straight from PSUM with fused 1/sqrt(D) scale -> P^T bf16
      PV as lhsT=V_aug (stationary), rhs=P^T wide-N, causal-trimmed column
      ranges accumulating in PSUM; row 64 = softmax denominator
      normalize along q: DVE reciprocal -> GpSimd partition_broadcast ->
      one DVE multiply writing attn^T[hd, q] bf16
    Wo matmuls for ss's four 128-row output tiles + DMA out fp32 partial
"""

import sys

if "/opt/trn_rl_repo" not in sys.path:
    sys.path.insert(0, "/opt/trn_rl_repo")

import numpy as np
import ml_dtypes

import concourse.bass as bass
from concourse import bacc
import concourse.mybir as mybir
from concourse.tile import TileContext
from concourse.masks import make_identity

BF16 = mybir.dt.bfloat16
F32 = mybir.dt.float32

B, S, HID = 2, 2048, 2048
H, HKV, D = 32, 8, 64
NCORES = 8
HQ = H // NCORES          # q heads per core (4)
HD = HQ * D               # 256: per-core attn feature dim
SCALE = D ** -0.5
SSUP = 512                # q supertile width
NEG = -1e9


def build_nc(b_sz=B, s_sz=S, hid=HID):
    """Build the per-core Bass program. Parameterized for small-sim testing."""
    C = hid // 128            # contraction chunks
    n_st = s_sz // 128        # 128-tiles along s
    sup = min(SSUP, s_sz)
    n_sup = s_sz // sup
    n_qt = sup // 128         # q-tiles per supertile
    n_cs = hid // 512         # 512-wide output column chunks

    nc = bacc.Bacc()
    hsT = nc.dram_tensor("hsT", [hid, b_sz * s_sz], BF16, kind="ExternalInput")
    wq = nc.dram_tensor("wq", [hid, HQ * D], BF16, kind="ExternalInput")
    wkv = nc.dram_tensor("wkv", [hid, 128], BF16, kind="ExternalInput")
    wo = nc.dram_tensor("wo", [HD, hid], BF16, kind="ExternalInput")
    cos2 = nc.dram_tensor("cos2", [128, s_sz], F32, kind="ExternalInput")
    sinx = nc.dram_tensor("sinx", [128, s_sz], F32, kind="ExternalInput")
    maskd = nc.dram_tensor("maskd", [128, 128], F32, kind="ExternalInput")
    out = nc.dram_tensor("out", [b_sz * s_sz, hid], F32, kind="ExternalOutput")

    hsT_v = hsT.rearrange("(co p) n -> p co n", p=128)
    wq_v = wq.rearrange("(co p) m -> p co m", p=128)
    wkv_v = wkv.rearrange("(co p) m -> p co m", p=128)
    wo_v = wo.rearrange("(j p) n -> p j n", p=128)

    with TileContext(nc) as tc:
        with (
            tc.tile_pool(name="const", bufs=1) as cpool,
            tc.tile_pool(name="hst", bufs=2) as hpool,
            tc.tile_pool(name="perb", bufs=2) as bpool,
            tc.tile_pool(name="pt", bufs=6) as ptpool,
            tc.tile_pool(name="work", bufs=2) as wpool,
            tc.tile_pool(name="outsb", bufs=2) as opool,
            tc.tile_pool(name="psum_qk", bufs=5, space="PSUM") as qkpool,
            tc.tile_pool(name="psum_pv", bufs=2, space="PSUM") as pvpool,
            tc.tile_pool(name="psum_tp", bufs=1, space="PSUM") as tppool,
        ):
            # ---- constants ----
            wq_t = cpool.tile([128, C, HQ * D], BF16, tag="wq")
            nc.sync.dma_start(wq_t[:], wq_v[:])
            wkv_t = cpool.tile([128, C, 128], BF16, tag="wkv")
            nc.sync.dma_start(wkv_t[:], wkv_v[:])
            wo_t = cpool.tile([128, HD // 128, hid], BF16, tag="wo")
            nc.sync.dma_start(wo_t[:], wo_v[:])
            cos_t = cpool.tile([128, s_sz], F32, tag="cos")
            nc.sync.dma_start(cos_t[:], cos2[:])
            sin_t = cpool.tile([128, s_sz], F32, tag="sin")
            nc.sync.dma_start(sin_t[:], sinx[:])
            mask_t = cpool.tile([128, 128], F32, tag="mask")
            nc.sync.dma_start(mask_t[:], maskd[:])
            ident = cpool.tile([128, 128], BF16, tag="ident")
            make_identity(nc, ident[:])

            def rope(dst, psum, s0, w, rows):
                """dst[bf16 SBUF [rows,w]] = RoPE(psum[:rows,:w]), tables at
                cols s0:s0+w. rows=64 (K) or 128 (2 stacked q-heads);
                out = psum*cos + shift32(psum)*sinx (sinx sign-folded)."""
                u = wpool.tile([128, sup], F32, tag="rope_u")
                t = wpool.tile([128, sup], F32, tag="rope_t")
                nc.vector.tensor_tensor(
                    u[:rows, :w], psum[:rows, :w], cos_t[:rows, s0:s0 + w],
                    mybir.AluOpType.mult)
                for o in range(0, rows, 64):
                    nc.vector.tensor_tensor(
                        t[o:o + 32, :w], psum[o + 32:o + 64, :w],
                        sin_t[o:o + 32, s0:s0 + w], mybir.AluOpType.mult)
                    nc.vector.tensor_tensor(
                        t[o + 32:o + 64, :w], psum[o:o + 32, :w],
                        sin_t[o + 32:o + 64, s0:s0 + w], mybir.AluOpType.mult)
                nc.vector.tensor_tensor(
                    dst, u[:rows, :w], t[:rows, :w], mybir.AluOpType.add)

            for b in range(b_sz):
                qt_b = bpool.tile([128, HQ // 2, s_sz], BF16, tag="qt")
                # K^T zero-padded to full 128 contraction rows, two variants:
                # variant 0 = [K^T; 0] (contracts q-heads at partitions 0:64),
                # variant 1 = [0; K^T] (heads at 64:128). Full-K matmuls keep
                # the PE HAM at k=8/8 (K=64 mms drop it to half clock).
                kt_b = bpool.tile([128, 2, s_sz], BF16, tag="kt")
                vt_b = bpool.tile([64, s_sz], BF16, tag="vt")
                # V padded to 128 lhsT columns: 0:64 = V, 64 = ones (softmax
                # denominator row), 65:128 = zeros (full-M for HAM).
                v_b = bpool.tile([128, n_st, 128], BF16, tag="v")
                attnT_b = bpool.tile([128, HD // 128, s_sz], BF16, tag="attnT")
                nc.vector.memset(v_b[:, :, 64:65], 1.0)
                nc.vector.memset(v_b[:, :, 65:128], 0.0)
                nc.vector.memset(kt_b[64:128, 0, :], 0.0)
                nc.vector.memset(kt_b[0:64, 1, :], 0.0)

                # ---- projections (all supertiles first: RoPE latency is off
                # the attention critical path, attention(qs) can start as soon
                # as super qs is projected) ----
                for ss in range(n_sup):
                    s0 = ss * sup
                    nbase = b * s_sz + s0
                    hst = hpool.tile([128, C, sup], BF16, tag="hst")
                    nc.sync.dma_start(hst[:], hsT_v[:, :, nbase:nbase + sup])
                    psqs = [qkpool.tile([128, sup], F32, tag="qk",
                                        name=f"psq{i}") for i in range(2)]
                    for cc in range(C):
                        for hp in range(HQ // 2):
                            nc.tensor.matmul(
                                psqs[hp][:],
                                wq_t[:, cc, hp * 128:(hp + 1) * 128],
                                hst[:, cc, :],
                                start=(cc == 0), stop=(cc == C - 1))
                    pskv = qkpool.tile([128, sup], F32, tag="qk")
                    for cc in range(C):
                        nc.tensor.matmul(
                            pskv[:], wkv_t[:, cc, :], hst[:, cc, :],
                            start=(cc == 0), stop=(cc == C - 1))
                    for hp in range(HQ // 2):
                        rope(qt_b[:, hp, s0:s0 + sup], psqs[hp], s0, sup, 128)
                    rope(kt_b[:64, 0, s0:s0 + sup], pskv[:], s0, sup, 64)
                    nc.vector.tensor_copy(
                        kt_b[64:128, 1, s0:s0 + sup], kt_b[:64, 0, s0:s0 + sup])
                    nc.vector.tensor_copy(
                        vt_b[:, s0:s0 + sup], pskv[64:128, :])
                    for st4 in range(n_qt):
                        st = ss * n_qt + st4
                        pst = tppool.tile([128, 128], BF16, tag="tp")
                        nc.tensor.transpose(
                            pst[:, :64], vt_b[:, st * 128:(st + 1) * 128],
                            ident[:64, :64])
                        nc.vector.tensor_copy(v_b[:, st, :64], pst[:, :64])

                # ---- attention ----
                for ss in range(n_sup):
                    s0 = ss * sup
                    n_kt = (ss + 1) * n_qt
                    for hp in range(HQ // 2):
                        heads = (2 * hp, 2 * hp + 1)
                        psvs = [pvpool.tile([128, sup], F32, tag="pv",
                                            name=f"psv{i}")
                                for i in range(2)]
                        for kt in range(n_kt):
                            k0 = kt * 128
                            dq = max(0, k0 - s0)
                            w = sup - dq
                            pt = ptpool.tile([128, 2, sup], BF16, tag="pt")
                            for sub, h in enumerate(heads):
                                qh = qt_b[:, h // 2, :]
                                kth = kt_b[:, h % 2, :]
                                ps = qkpool.tile([128, sup], F32, tag="qk")
                                if k0 < s0:
                                    nc.tensor.matmul(
                                        ps[:], kth[:, k0:k0 + 128],
                                        qh[:, s0:s0 + sup],
                                        start=True, stop=True)
                                else:
                                    nc.tensor.matmul(
                                        ps[:, 0:128],
                                        kth[:, k0:k0 + 128],
                                        qh[:, k0:k0 + 128],
                                        start=True, stop=True)
                                    if w > 128:
                                        nc.tensor.matmul(
                                            ps[:, 128:w],
                                            kth[:, k0:k0 + 128],
                                            qh[:, k0 + 128:s0 + sup],
                                            start=True, stop=True)
                                if k0 >= s0:
                                    nc.vector.tensor_tensor(
                                        ps[:, 0:128], ps[:, 0:128], mask_t[:],
                                        mybir.AluOpType.add)
                                nc.scalar.activation(
                                    pt[:, sub, dq:dq + w], ps[:, 0:w],
                                    mybir.ActivationFunctionType.Exp,
                                    scale=SCALE)
                            for sub in range(2):
                                nc.tensor.matmul(
                                    psvs[sub][:, dq:sup],
                                    v_b[:, kt, :],
                                    pt[:, sub, dq:sup],
                                    start=(kt == 0), stop=(kt == n_kt - 1),
                                    skip_group_check=True)
                        for sub, h in enumerate(heads):
                            psv = psvs[sub]
                            # 1/Z along the free dim: DVE reciprocal on a
                            # [1,512] row is single-lane serial (~3.3us), so
                            # compute exp(-ln Z) on ScalarE instead.
                            lnz = wpool.tile([1, sup], F32, tag="lnz")
                            nc.scalar.activation(
                                lnz[:], psv[64:65, :],
                                mybir.ActivationFunctionType.Ln)
                            recip = wpool.tile([1, sup], F32, tag="recip")
                            nc.scalar.activation(
                                recip[:], lnz[:],
                                mybir.ActivationFunctionType.Exp, scale=-1.0)
                            bcast = wpool.tile([64, sup], F32, tag="bcast")
                            nc.gpsimd.partition_broadcast(bcast[:], recip[:])
                            o = (h % 2) * 64
                            nc.vector.tensor_tensor(
                                attnT_b[o:o + 64, h // 2, s0:s0 + sup],
                                psv[0:64, :], bcast[:], mybir.AluOpType.mult)

                # ---- output projection ----
                for st in range(n_st):
                    osb = opool.tile([128, hid], F32, tag="osb")
                    for cs in range(n_cs):
                        pso = qkpool.tile([128, sup], F32, tag="qk")
                        for j in range(HD // 128):
                            nc.tensor.matmul(
                                pso[:, :512],
                                attnT_b[:, j, st * 128:(st + 1) * 128],
                                wo_t[:, j, cs * 512:(cs + 1) * 512],
                                start=(j == 0),
                                stop=(j == HD // 128 - 1))
                        nc.vector.tensor_copy(
                            osb[:, cs * 512:(cs + 1) * 512], pso[:, :512])
                    row = b * s_sz + st * 128
                    nc.sync.dma_start(out[row:row + 128, :], osb[:])
    nc.compile()
    return nc


def _rope_tables_np(seq_len, dim, base=10000.0):
    inv_freq = 1.0 / (base ** (np.arange(0, dim, 2, dtype=np.float32) / dim))
    t = np.arange(seq_len, dtype=np.float32)
    freqs = np.outer(t, inv_freq)
    emb = np.concatenate([freqs, freqs], axis=-1)
    return np.cos(emb), np.sin(emb)


def host_prep(hidden_states, cos, sin, Wq, Wk, Wv, Wo, s_sz=None, hid=None):
    """Slice/transposes/casts -> per-core input maps."""
    b_sz = hidden_states.shape[0]
    s_sz = s_sz or hidden_states.shape[1]
    hid = hid or hidden_states.shape[2]
    bf = ml_dtypes.bfloat16

    hsT = np.ascontiguousarray(
        hidden_states.reshape(b_sz * s_sz, hid).T).astype(bf)

    cosT = np.asarray(cos, np.float32).T          # [64, S]
    sinT = np.asarray(sin, np.float32).T
    cos2 = np.concatenate([cosT, cosT], axis=0)   # [128, S]
    sinx = np.concatenate(
        [-sinT[:32], sinT[32:64], -sinT[:32], sinT[32:64]], axis=0)
    cos2 = np.ascontiguousarray(cos2, dtype=np.float32)
    sinx = np.ascontiguousarray(sinx, dtype=np.float32)

    kk, qq = np.meshgrid(np.arange(128), np.arange(128), indexing="ij")
    maskd = np.where(kk <= qq, 0.0, NEG).astype(np.float32)

    in_maps = []
    for c in range(NCORES):
        wq_c = np.ascontiguousarray(Wq[:, c * HD:(c + 1) * HD]).astype(bf)
        wkv_c = np.concatenate(
            [Wk[:, c * D:(c + 1) * D], Wv[:, c * D:(c + 1) * D]], axis=1
        ).astype(bf)
        wo_c = np.ascontiguousarray(Wo[c * HD:(c + 1) * HD, :]).astype(bf)
        in_maps.append({
            "hsT": hsT, "wq": wq_c, "wkv": np.ascontiguousarray(wkv_c),
            "wo": wo_c, "cos2": cos2, "sinx": sinx, "maskd": maskd,
        })
    return in_maps


def kernel_run(hidden_states, cos, sin, attention_mask, Wq, Wk, Wv, Wo,
               **spmd_kwargs):
    from concourse.bass_utils import run_bass_kernel_spmd

    hidden_states = np.asarray(hidden_states, np.float32)
    in_maps = host_prep(hidden_states, cos, sin,
                        np.asarray(Wq, np.float32), np.asarray(Wk, np.float32),
                        np.asarray(Wv, np.float32), np.asarray(Wo, np.float32))
    nc = build_nc()
    res = run_bass_kernel_spmd(nc, in_maps, core_ids=list(range(NCORES)),
                               **spmd_kwargs)
    acc = np.zeros((B * S, HID), np.float64)
    for r in res.results:
        acc += r["out"].astype(np.float64)
    return acc.reshape(B, S, HID).astype(np.float32), res


def kernel(hidden_states, cos, sin, attention_mask, Wq, Wk, Wv, Wo):
    out, _ = kernel_run(hidden_states, cos, sin, attention_mask,
                        Wq, Wk, Wv, Wo)
    return out


if __name__ == "__main__":
    pass


# revision 22
# speedup vs baseline: 1.3563x; 1.1132x over previous
"""Trainium2 Bass kernel for GQA attention (nn_Attention_43181601194655).

Full module: hidden [B,S,HID] -> Wq/Wk/Wv proj -> RoPE -> causal GQA attention
-> Wo proj. Sharded tensor-parallel over heads across 8 NeuronCores:
core c owns q-heads [4c..4c+4) and kv-head c (Wq/Wk/Wv column slices, Wo row
slice). Each core computes a full-shape partial output; the host sums the 8
partials (the row-parallel Wo reduction).

v2 pipeline (single pass over q supertiles, per batch):
  for each 512-wide s-supertile ss:
    project Q^T/K^T/V^T for ss (hidden^T streamed in bf16), RoPE on-chip
    PE-transpose V^T -> V tiles (with ones column for softmax denominators)
    attention for q-super ss (all 4 heads, keys 0..ss complete by now):
      S^T[k,q] matmuls head-paired on PE row groups (bases 0/64), exp on
      ScalarE straight from PSUM with fused 1/sqrt(D) scale -> P^T bf16
      PV as lhsT=V_aug (stationary), rhs=P^T wide-N, causal-trimmed column
      ranges accumulating in PSUM; row 64 = softmax denominator
      normalize along q: DVE reciprocal -> GpSimd partition_broadcast ->
      one DVE multiply writing attn^T[hd, q] bf16
    Wo matmuls for ss's four 128-row output tiles + DMA out fp32 partial
"""

import sys

if "/opt/trn_rl_repo" not in sys.path:
    sys.path.insert(0, "/opt/trn_rl_repo")

import numpy as np
import ml_dtypes

import concourse.bass as bass
from concourse import bacc
import concourse.mybir as mybir
from concourse.tile import TileContext
from concourse.masks import make_identity

BF16 = mybir.dt.bfloat16
F32 = mybir.dt.float32

B, S, HID = 2, 2048, 2048
H, HKV, D = 32, 8, 64
NCORES = 8
HQ = H // NCORES          # q heads per core (4)
HD = HQ * D               # 256: per-core attn feature dim
SCALE = D ** -0.5
SSUP = 512                # q supertile width
NEG = -1e9


def build_nc(b_sz=B, s_sz=S, hid=HID):
    """Build the per-core Bass program. Parameterized for small-sim testing."""
    C = hid // 128            # contraction chunks
    n_st = s_sz // 128        # 128-tiles along s
    sup = min(SSUP, s_sz)
    n_sup = s_sz // sup
    n_qt = sup // 128         # q-tiles per supertile
    n_cs = hid // 512         # 512-wide output column chunks

    nc = bacc.Bacc()
    hsT = nc.dram_tensor("hsT", [hid, b_sz * s_sz], BF16, kind="ExternalInput")
    wq = nc.dram_tensor("wq", [hid, HQ * D], BF16, kind="ExternalInput")
    wkv = nc.dram_tensor("wkv", [hid, 128], BF16, kind="ExternalInput")
    wo = nc.dram_tensor("wo", [HD, hid], BF16, kind="ExternalInput")
    cos2 = nc.dram_tensor("cos2", [128, s_sz], F32, kind="ExternalInput")
    sinx = nc.dram_tensor("sinx", [128, s_sz], F32, kind="ExternalInput")
    maskd = nc.dram_tensor("maskd", [128, 128], F32, kind="ExternalInput")
    out = nc.dram_tensor("out", [b_sz * s_sz, hid], F32, kind="ExternalOutput")

    hsT_v = hsT.rearrange("(co p) n -> p co n", p=128)
    wq_v = wq.rearrange("(co p) m -> p co m", p=128)
    wkv_v = wkv.rearrange("(co p) m -> p co m", p=128)
    wo_v = wo.rearrange("(j p) n -> p j n", p=128)

    with TileContext(nc) as tc:
        with (
            tc.tile_pool(name="const", bufs=1) as cpool,
            tc.tile_pool(name="hst", bufs=2) as hpool,
            tc.tile_pool(name="perb", bufs=2) as bpool,
            tc.tile_pool(name="pt", bufs=6) as ptpool,
            tc.tile_pool(name="work", bufs=2) as wpool,
            tc.tile_pool(name="outsb", bufs=2) as opool,
            tc.tile_pool(name="psum_qk", bufs=5, space="PSUM") as qkpool,
            tc.tile_pool(name="psum_pv", bufs=3, space="PSUM") as pvpool,
        ):
            # ---- constants ----
            wq_t = cpool.tile([128, C, HQ * D], BF16, tag="wq")
            nc.sync.dma_start(wq_t[:], wq_v[:])
            wkv_t = cpool.tile([128, C, 128], BF16, tag="wkv")
            nc.sync.dma_start(wkv_t[:], wkv_v[:])
            wo_t = cpool.tile([128, HD // 128, hid], BF16, tag="wo")
            nc.sync.dma_start(wo_t[:], wo_v[:])
            cos_t = cpool.tile([128, s_sz], F32, tag="cos")
            nc.sync.dma_start(cos_t[:], cos2[:])
            sin_t = cpool.tile([128, s_sz], F32, tag="sin")
            nc.sync.dma_start(sin_t[:], sinx[:])
            mask_t = cpool.tile([128, 128], F32, tag="mask")
            nc.sync.dma_start(mask_t[:], maskd[:])
            ident = cpool.tile([128, 128], BF16, tag="ident")
            make_identity(nc, ident[:])

            def rope(dst, psum, s0, w, rows):
                """dst[bf16 SBUF [rows,w]] = RoPE(psum[:rows,:w]), tables at
                cols s0:s0+w. rows=64 (K) or 128 (2 stacked q-heads);
                out = psum*cos + shift32(psum)*sinx (sinx sign-folded)."""
                u = wpool.tile([128, sup], F32, tag="rope_u")
                t = wpool.tile([128, sup], F32, tag="rope_t")
                nc.vector.tensor_tensor(
                    u[:rows, :w], psum[:rows, :w], cos_t[:rows, s0:s0 + w],
                    mybir.AluOpType.mult)
                for o in range(0, rows, 64):
                    nc.vector.tensor_tensor(
                        t[o:o + 32, :w], psum[o + 32:o + 64, :w],
                        sin_t[o:o + 32, s0:s0 + w], mybir.AluOpType.mult)
                    nc.vector.tensor_tensor(
                        t[o + 32:o + 64, :w], psum[o:o + 32, :w],
                        sin_t[o + 32:o + 64, s0:s0 + w], mybir.AluOpType.mult)
                nc.vector.tensor_tensor(
                    dst, u[:rows, :w], t[:rows, :w], mybir.AluOpType.add)

            for b in range(b_sz):
                qt_b = bpool.tile([128, HQ // 2, s_sz], BF16, tag="qt")
                # K^T zero-padded to full 128 contraction rows, two variants:
                # variant 0 = [K^T; 0] (contracts q-heads at partitions 0:64),
                # variant 1 = [0; K^T] (heads at 64:128). Full-K matmuls keep
                # the PE HAM at k=8/8 (K=64 mms drop it to half clock).
                kt_b = bpool.tile([128, 2, s_sz], BF16, tag="kt")
                vt_b = bpool.tile([64, s_sz], BF16, tag="vt")
                # V padded to 128 lhsT columns: 0:64 = V, 64 = ones (softmax
                # denominator row), 65:128 = zeros (full-M for HAM).
                v_b = bpool.tile([128, n_st, 128], BF16, tag="v")
                attnT_b = bpool.tile([128, HD // 128, s_sz], BF16, tag="attnT")
                nc.vector.memset(v_b[:, :, 64:65], 1.0)
                nc.vector.memset(v_b[:, :, 65:128], 0.0)
                nc.vector.memset(kt_b[64:128, 0, :], 0.0)
                nc.vector.memset(kt_b[0:64, 1, :], 0.0)

                # ---- projections (all supertiles first: RoPE latency is off
                # the attention critical path, attention(qs) can start as soon
                # as super qs is projected) ----
                for ss in range(n_sup):
                    s0 = ss * sup
                    nbase = b * s_sz + s0
                    hst = hpool.tile([128, C, sup], BF16, tag="hst")
                    nc.sync.dma_start(hst[:], hsT_v[:, :, nbase:nbase + sup])
                    psqs = [qkpool.tile([128, sup], F32, tag="qk",
                                        name=f"psq{i}") for i in range(2)]
                    for cc in range(C):
                        for hp in range(HQ // 2):
                            nc.tensor.matmul(
                                psqs[hp][:],
                                wq_t[:, cc, hp * 128:(hp + 1) * 128],
                                hst[:, cc, :],
                                start=(cc == 0), stop=(cc == C - 1))
                    pskv = qkpool.tile([128, sup], F32, tag="qk")
                    for cc in range(C):
                        nc.tensor.matmul(
                            pskv[:], wkv_t[:, cc, :], hst[:, cc, :],
                            start=(cc == 0), stop=(cc == C - 1))
                    for hp in range(HQ // 2):
                        rope(qt_b[:, hp, s0:s0 + sup], psqs[hp], s0, sup, 128)
                    rope(kt_b[:64, 0, s0:s0 + sup], pskv[:], s0, sup, 64)
                    nc.vector.tensor_copy(
                        kt_b[64:128, 1, s0:s0 + sup], kt_b[:64, 0, s0:s0 + sup])
                    nc.vector.tensor_copy(
                        vt_b[:, s0:s0 + sup], pskv[64:128, :])
                    for st4 in range(n_qt):
                        st = ss * n_qt + st4
                        pst = qkpool.tile([128, 128], BF16, tag="qk",
                                          name="pst")
                        nc.tensor.transpose(
                            pst[:, :64], vt_b[:, st * 128:(st + 1) * 128],
                            ident[:64, :64])
                        nc.vector.tensor_copy(v_b[:, st, :64], pst[:, :64])

                # ---- attention ----
                for ss in range(n_sup):
                    s0 = ss * sup
                    n_kt = (ss + 1) * n_qt
                    for hp in range(HQ // 2):
                        heads = (2 * hp, 2 * hp + 1)
                        psvs = [pvpool.tile([128, sup], F32, tag="pv",
                                            name=f"psv{i}")
                                for i in range(2)]
                        for kt in range(n_kt):
                            k0 = kt * 128
                            dq = max(0, k0 - s0)
                            w = sup - dq
                            pt = ptpool.tile([128, 2, sup], BF16, tag="pt")
                            for sub, h in enumerate(heads):
                                qh = qt_b[:, h // 2, :]
                                kth = kt_b[:, h % 2, :]
                                ps = qkpool.tile([128, sup], F32, tag="qk")
                                if k0 < s0:
                                    nc.tensor.matmul(
                                        ps[:], kth[:, k0:k0 + 128],
                                        qh[:, s0:s0 + sup],
                                        start=True, stop=True)
                                else:
                                    nc.tensor.matmul(
                                        ps[:, 0:128],
                                        kth[:, k0:k0 + 128],
                                        qh[:, k0:k0 + 128],
                                        start=True, stop=True)
                                    if w > 128:
                                        nc.tensor.matmul(
                                            ps[:, 128:w],
                                            kth[:, k0:k0 + 128],
                                            qh[:, k0 + 128:s0 + sup],
                                            start=True, stop=True)
                                if k0 >= s0:
                                    nc.vector.tensor_tensor(
                                        ps[:, 0:128], ps[:, 0:128], mask_t[:],
                                        mybir.AluOpType.add)
                                nc.scalar.activation(
                                    pt[:, sub, dq:dq + w], ps[:, 0:w],
                                    mybir.ActivationFunctionType.Exp,
                                    scale=SCALE)
                            for sub in range(2):
                                nc.tensor.matmul(
                                    psvs[sub][:, dq:sup],
                                    v_b[:, kt, :],
                                    pt[:, sub, dq:sup],
                                    start=(kt == 0), stop=(kt == n_kt - 1),
                                    skip_group_check=True)
                        for sub, h in enumerate(heads):
                            psv = psvs[sub]
                            zrow = wpool.tile([1, sup], F32, tag="zrow")
                            nc.vector.tensor_copy(zrow[:], psv[64:65, :])
                            recip = wpool.tile([1, sup], F32, tag="recip")
                            nc.vector.reciprocal_approx_fast(
                                recip[:], zrow[:])
                            bcast = wpool.tile([64, sup], F32, tag="bcast")
                            nc.gpsimd.partition_broadcast(bcast[:], recip[:])
                            o = (h % 2) * 64
                            nc.vector.tensor_tensor(
                                attnT_b[o:o + 64, h // 2, s0:s0 + sup],
                                psv[0:64, :], bcast[:], mybir.AluOpType.mult)

                # ---- output projection ----
                for st in range(n_st):
                    osb = opool.tile([128, hid], F32, tag="osb")
                    for cs in range(n_cs):
                        pso = qkpool.tile([128, sup], F32, tag="qk")
                        for j in range(HD // 128):
                            nc.tensor.matmul(
                                pso[:, :512],
                                attnT_b[:, j, st * 128:(st + 1) * 128],
                                wo_t[:, j, cs * 512:(cs + 1) * 512],
                                start=(j == 0),
                                stop=(j == HD // 128 - 1))
                        if cs % 2 == 0:
                            nc.vector.tensor_copy(
                                osb[:, cs * 512:(cs + 1) * 512], pso[:, :512])
                        else:
                            nc.scalar.copy(
                                osb[:, cs * 512:(cs + 1) * 512], pso[:, :512])
                    row = b * s_sz + st * 128
                    nc.sync.dma_start(out[row:row + 128, :], osb[:])
    nc.compile()
    return nc


def _rope_tables_np(seq_len, dim, base=10000.0):
    inv_freq = 1.0 / (base ** (np.arange(0, dim, 2, dtype=np.float32) / dim))
    t = np.arange(seq_len, dtype=np.float32)
    freqs = np.outer(t, inv_freq)
    emb = np.concatenate([freqs, freqs], axis=-1)
    return np.cos(emb), np.sin(emb)


def host_prep(hidden_states, cos, sin, Wq, Wk, Wv, Wo, s_sz=None, hid=None):
    """Slice/transposes/casts -> per-core input maps."""
    b_sz = hidden_states.shape[0]
    s_sz = s_sz or hidden_states.shape[1]
    hid = hid or hidden_states.shape[2]
    bf = ml_dtypes.bfloat16

    hsT = np.ascontiguousarray(
        hidden_states.reshape(b_sz * s_sz, hid).T).astype(bf)

    cosT = np.asarray(cos, np.float32).T          # [64, S]
    sinT = np.asarray(sin, np.float32).T
    cos2 = np.concatenate([cosT, cosT], axis=0)   # [128, S]
    sinx = np.concatenate(
        [-sinT[:32], sinT[32:64], -sinT[:32], sinT[32:64]], axis=0)
    cos2 = np.ascontiguousarray(cos2, dtype=np.float32)
    sinx = np.ascontiguousarray(sinx, dtype=np.float32)

    kk, qq = np.meshgrid(np.arange(128), np.arange(128), indexing="ij")
    maskd = np.where(kk <= qq, 0.0, NEG).astype(np.float32)

    in_maps = []
    for c in range(NCORES):
        wq_c = np.ascontiguousarray(Wq[:, c * HD:(c + 1) * HD]).astype(bf)
        wkv_c = np.concatenate(
            [Wk[:, c * D:(c + 1) * D], Wv[:, c * D:(c + 1) * D]], axis=1
        ).astype(bf)
        wo_c = np.ascontiguousarray(Wo[c * HD:(c + 1) * HD, :]).astype(bf)
        in_maps.append({
            "hsT": hsT, "wq": wq_c, "wkv": np.ascontiguousarray(wkv_c),
            "wo": wo_c, "cos2": cos2, "sinx": sinx, "maskd": maskd,
        })
    return in_maps


def kernel_run(hidden_states, cos, sin, attention_mask, Wq, Wk, Wv, Wo,
               **spmd_kwargs):
    from concourse.bass_utils import run_bass_kernel_spmd

    hidden_states = np.asarray(hidden_states, np.float32)
    in_maps = host_prep(hidden_states, cos, sin,
                        np.asarray(Wq, np.float32), np.asarray(Wk, np.float32),
                        np.asarray(Wv, np.float32), np.asarray(Wo, np.float32))
    nc = build_nc()
    res = run_bass_kernel_spmd(nc, in_maps, core_ids=list(range(NCORES)),
                               **spmd_kwargs)
    acc = np.zeros((B * S, HID), np.float64)
    for r in res.results:
        acc += r["out"].astype(np.float64)
    return acc.reshape(B, S, HID).astype(np.float32), res


def kernel(hidden_states, cos, sin, attention_mask, Wq, Wk, Wv, Wo):
    out, _ = kernel_run(hidden_states, cos, sin, attention_mask,
                        Wq, Wk, Wv, Wo)
    return out


if __name__ == "__main__":
    pass
